# revision 3
# baseline (speedup 1.0000x reference)
"""Trainium2 Bass kernel for GuidedFilterHR (bilateral-weighted guided filter).

Strategy (per NeuronCore, 8 cores, column-sharded):
  - Layout: column-major tiles [partition = image column, free = image row].
    Core k owns columns [128k, 128k+128); all 512 rows (no row halo needed).
  - 5x5 zero-padded box filter: vertical pass on DVE/Pool (free-dim shifts),
    horizontal pass on TensorE as Toeplitz-band matmuls (incl. column halo).
  - Bilateral weighted sums, centered in x / uncentered in y:
      den = sum w, M1 = sum w*d, M2 = sum w*d^2,
      N = sum w*ybs, C2 = sum w*d*ybs        (d = xb_shifted - xb_center)
    give  mean_y = N/den, A = cov/(var+1e-6),
      out_B = N/den - A*(M1/den),
      cov = C2/den - (M1/den)*(N/den), var = M2/den - (M1/den)^2.
    Algebraically identical to the reference guided filter output.
  - w = exp(-c*d^2 + ln(spatial) + border_mask) with the bias table encoding
    both the spatial weight and image-border masking (-50 => w=0); row
    borders handled by restricting accumulation row ranges per dy.
  - All per-offset elementwise tensors are bf16 (packed => DVE 2x mode);
    accumulation in fp32 PSUM via TensorE identity matmuls.  The square and
    the 4 products per offset are greedily balanced across DVE/Pool/Act by a
    static cost model (DVE 290ns, Pool 1111ns, Act 612ns per [128,512] op).
  - The reference's width chunking (CHUNK=64, OVL=10, TRIM=5) is equivalent
    to full-width processing because kept pixels' windows never cross chunk
    borders.
"""

import math
import numpy as np

# ---------------------------------------------------------------------------
# Problem constants (hardcoded; kernel.py must be self-contained)
# ---------------------------------------------------------------------------
M, N = 512, 1024          # image rows, cols
NCORES = 8
CW = N // NCORES          # 128 columns per core
HW_ = 8                   # halo width stored each side (box needs 7; 8 for pad)
RB = 5                    # bilateral B radius (11x11)
RD = 2                    # bilateral D radius (5x5)
DEN_B = (121 / 4.0) ** 2  # spatial denom for 11x11
DEN_D = (25 / 4.0) ** 2   # spatial denom for 5x5
MASK_BIAS = -50.0

_PROGRAM_CACHE = {}


class _Balance:
    """Greedy static load balancer for elementwise ops across DVE/Pool/Act."""
    DVE_BF16 = 290.0
    DVE_F32 = 560.0
    POOL = 1111.0
    ACT = 612.0

    def __init__(self, nc):
        self.nc = nc
        self.t = {"dve": 0.0, "pool": 0.0, "act": 0.0}

    def eng(self, name):
        return {"dve": self.nc.vector, "pool": self.nc.gpsimd,
                "act": self.nc.scalar}[name]

    def pick_tt(self, f32=False, allow_act=False):
        """Pick engine for a tensor_tensor-ish op; returns (engine, name)."""
        cand = {"dve": self.DVE_F32 if f32 else self.DVE_BF16,
                "pool": self.POOL}
        if allow_act:
            cand["act"] = self.ACT
        best = min(cand, key=lambda k: self.t[k] + cand[k])
        self.t[best] += cand[best]
        return self.eng(best), best

    def charge(self, name, ns):
        self.t[name] += ns


def _build_program():
    import concourse.bacc as bacc
    import concourse.tile as tile
    import concourse.mybir as mybir
    from concourse import bass

    f32 = mybir.dt.float32
    bf16 = mybir.dt.bfloat16
    Alu = mybir.AluOpType
    Act = mybir.ActivationFunctionType

    nc = bacc.Bacc("TRN2", target_bir_lowering=False, debug=False,
                   num_devices=NCORES)

    # ---------------- DRAM I/O ----------------
    # xym: [CW, 2M] = [xT | yT] main columns; xyh: [2*HW_, 2M] halo columns
    d_xym = nc.dram_tensor("xym", [CW, 2 * M], f32, kind="ExternalInput").ap()
    d_xyh = nc.dram_tensor("xyh", [2 * HW_, 2 * M], f32, kind="ExternalInput").ap()
    d_biasB = nc.dram_tensor("biasB", [CW, 121], f32, kind="ExternalInput").ap()
    d_biasD = nc.dram_tensor("biasD", [CW, 25], f32, kind="ExternalInput").ap()
    d_sqc = nc.dram_tensor("sqc", [CW, 1], f32, kind="ExternalInput").ap()
    d_negc = nc.dram_tensor("negc", [CW, 1], f32, kind="ExternalInput").ap()
    d_ident = nc.dram_tensor("ident", [CW, CW], bf16, kind="ExternalInput").ap()
    d_tmm = nc.dram_tensor("tmm", [CW, CW], f32, kind="ExternalInput").ap()
    d_thm = nc.dram_tensor("thm", [2 * HW_, CW], f32, kind="ExternalInput").ap()
    d_tmh = nc.dram_tensor("tmh", [CW, 2 * HW_], f32, kind="ExternalInput").ap()
    d_thh = nc.dram_tensor("thh", [2 * HW_, 2 * HW_], f32, kind="ExternalInput").ap()
    d_out = nc.dram_tensor("outT", [CW, M], f32, kind="ExternalOutput").ap()

    bal = None

    with tile.TileContext(nc) as tc:
        with tc.tile_pool(name="cst", bufs=1) as cst, \
             tc.tile_pool(name="per", bufs=1) as per, \
             tc.tile_pool(name="wrk", bufs=4) as wrk, \
             tc.tile_pool(name="ps", bufs=1, space="PSUM") as ps:

            bal = _Balance(nc)

            # ---------------- load constants + inputs ----------------
            xym = cst.tile([CW, 2 * M], f32, name="xym_s", tag="xym_s")
            xyh = cst.tile([2 * HW_, 2 * M], f32, name="xyh_s", tag="xyh_s")
            biasB = cst.tile([CW, 121], f32, name="biasB_s", tag="biasB_s")
            biasD = cst.tile([CW, 25], f32, name="biasD_s", tag="biasD_s")
            sqc = cst.tile([CW, 1], f32, name="sqc_s", tag="sqc_s")
            negc = cst.tile([CW, 1], f32, name="negc_s", tag="negc_s")
            ident = cst.tile([CW, CW], bf16, name="ident_s", tag="ident_s")
            tmm = cst.tile([CW, CW], f32, name="tmm_s", tag="tmm_s")
            thm = cst.tile([2 * HW_, CW], f32, name="thm_s", tag="thm_s")
            tmh = cst.tile([CW, 2 * HW_], f32, name="tmh_s", tag="tmh_s")
            thh = cst.tile([2 * HW_, 2 * HW_], f32, name="thh_s", tag="thh_s")
            for dst, src in [(xym, d_xym), (xyh, d_xyh),
                             (biasB, d_biasB), (biasD, d_biasD),
                             (sqc, d_sqc), (negc, d_negc), (ident, d_ident),
                             (tmm, d_tmm), (thm, d_thm),
                             (tmh, d_tmh), (thh, d_thh)]:
                nc.sync.dma_start(dst[:], src[:])

            # ---------------- 5x5 box: vertical pass (zero-padded) ----------
            # combined [x|y] tiles: vertical shifts stay within each M-segment
            vm = per.tile([CW, 2 * M], f32, name="vm", tag="vm")
            vh = per.tile([2 * HW_, 2 * M], f32, name="vh", tag="vh")

            def vbox(eng, dst, src):
                # dst[:, s*M+r] = sum_dy src[:, s*M+r+dy], per segment s
                eng.tensor_copy(dst[:], src[:])
                for dy in (-2, -1, 1, 2):
                    lo, hi = max(0, -dy), M - max(0, dy)
                    for s in (0, M):
                        eng.tensor_tensor(dst[:, s + lo:s + hi],
                                          dst[:, s + lo:s + hi],
                                          src[:, s + lo + dy:s + hi + dy],
                                          Alu.add)

            vbox(nc.vector, vm, xym)   # 9 f32 ops on DVE
            bal.charge("dve", 9 * 2 * bal.DVE_F32)
            vbox(nc.gpsimd, vh, xyh)   # 9 f32 ops on Pool (16-partition, cheap)
            bal.charge("pool", 9 * 0.3 * bal.POOL)

            # ---------------- 5x5 box: horizontal pass on PE ----------------
            # (Toeplitz band matrices carry the 1/25 factor)
            psXb = ps.tile([CW, M], f32, tag="a0", name="psXb")
            psYb = ps.tile([CW, M], f32, tag="a1", name="psYb")
            psXbh = ps.tile([2 * HW_, M], f32, tag="a2", name="psXbh")
            psYbh = ps.tile([2 * HW_, M], f32, tag="a3", name="psYbh")
            for (pm, ph, s) in [(psXb, psXbh, 0), (psYb, psYbh, M)]:
                nc.tensor.matmul(pm[:], tmm[:], vm[:, s:s + M],
                                 start=True, stop=False)
                nc.tensor.matmul(pm[:], thm[:], vh[:, s:s + M],
                                 start=False, stop=True)
                nc.tensor.matmul(ph[:], tmh[:], vm[:, s:s + M],
                                 start=True, stop=False)
                nc.tensor.matmul(ph[:], thh[:], vh[:, s:s + M],
                                 start=False, stop=True)

            # ---------------- evacuate box results + bf16 casts -------------
            Xb_f = per.tile([CW, M], f32, name="Xb_f", tag="Xb_f")
            yb_f = per.tile([CW, M], f32, name="yb_f", tag="yb_f")
            Xbh_f = per.tile([2 * HW_, M], f32, name="Xbh_f", tag="Xbh_f")
            ybh_f = per.tile([2 * HW_, M], f32, name="ybh_f", tag="ybh_f")
            B_b = per.tile([CW, 2 * M], bf16, name="B_b", tag="B_b")
            Bh_b = per.tile([2 * HW_, 2 * M], bf16, name="Bh_b", tag="Bh_b")
            nc.scalar.copy(Xb_f[:], psXb[:])
            nc.scalar.copy(yb_f[:], psYb[:])
            nc.scalar.copy(Xbh_f[:], psXbh[:])
            nc.scalar.copy(ybh_f[:], psYbh[:])
            nc.scalar.copy(B_b[:, 0:M], psXb[:])
            nc.scalar.copy(B_b[:, M:2 * M], psYb[:])
            nc.scalar.copy(Bh_b[:, 0:M], psXbh[:])
            nc.scalar.copy(Bh_b[:, M:2 * M], psYbh[:])
            bal.charge("act", 8 * bal.ACT)

            # ---------------- detail tensors ----------------
            # D_b = [xd | z] bf16, z = yd - xd;  xd_f kept f32 for assembly
            xd_f = per.tile([CW, M], f32, name="xd_f", tag="xd_f")
            yd_f = per.tile([CW, M], f32, name="yd_f", tag="yd_f")
            D_b = per.tile([CW, 2 * M], bf16, name="D_b", tag="D_b")
            Dh_b = per.tile([2 * HW_, 2 * M], bf16, name="Dh_b", tag="Dh_b")
            xdh_f = per.tile([2 * HW_, M], f32, name="xdh_f", tag="xdh_f")
            ydh_f = per.tile([2 * HW_, M], f32, name="ydh_f", tag="ydh_f")
            nc.vector.tensor_tensor(xd_f[:], xym[:, 0:M], Xb_f[:], Alu.subtract)
            nc.vector.tensor_tensor(yd_f[:], xym[:, M:2 * M], yb_f[:], Alu.subtract)
            nc.vector.tensor_copy(D_b[:, 0:M], xd_f[:])
            nc.vector.tensor_tensor(D_b[:, M:2 * M], yd_f[:], xd_f[:], Alu.subtract)
            bal.charge("dve", 4 * bal.DVE_F32)
            nc.gpsimd.tensor_tensor(xdh_f[:], xyh[:, 0:M], Xbh_f[:], Alu.subtract)
            nc.gpsimd.tensor_tensor(ydh_f[:], xyh[:, M:2 * M], ybh_f[:], Alu.subtract)
            nc.gpsimd.tensor_copy(Dh_b[:, 0:M], xdh_f[:])
            nc.gpsimd.tensor_tensor(Dh_b[:, M:2 * M], ydh_f[:], xdh_f[:], Alu.subtract)
            bal.charge("pool", 4 * 0.3 * bal.POOL)

            # ---------------- horizontal-shift materializations -------------
            # dst[p] = src(col c0+p+dx); cross-partition moves must go via DMA
            # (compute engines require 32-aligned start partitions).
            # Combined [x|y] (or [xd|z]) tiles: one DMA pair moves both maps.
            def hshift(dst, src_main, src_halo, dx):
                if dx > 0:
                    nc.sync.dma_start(dst[0:CW - dx, :], src_main[dx:CW, :])
                    nc.sync.dma_start(dst[CW - dx:CW, :],
                                      src_halo[HW_:HW_ + dx, :])
                else:
                    nc.sync.dma_start(dst[-dx:CW, :], src_main[0:CW + dx, :])
                    nc.sync.dma_start(dst[0:-dx, :],
                                      src_halo[HW_ + dx:HW_, :])

            BS = {0: B_b}
            for dx in range(-RB, RB + 1):
                if dx == 0:
                    continue
                t = per.tile([CW, 2 * M], bf16, name=f"bs_{dx + RB}")
                hshift(t, B_b, Bh_b, dx)
                BS[dx] = t
            DS = {0: D_b}
            for dx in range(-RD, RD + 1):
                if dx == 0:
                    continue
                t = per.tile([CW, 2 * M], bf16, name=f"ds_{dx + RD}")
                hshift(t, D_b, Dh_b, dx)
                DS[dx] = t

            # ---------------- PSUM accumulators ----------------
            den = ps.tile([CW, M], f32, tag="a0", name="acc_den")
            M1 = ps.tile([CW, M], f32, tag="a1", name="acc_m1")
            M2 = ps.tile([CW, M], f32, tag="a2", name="acc_m2")
            NN = ps.tile([CW, M], f32, tag="a3", name="acc_n")
            C2 = ps.tile([CW, M], f32, tag="a4", name="acc_c2")
            denD = ps.tile([CW, M], f32, tag="a5", name="acc_dend")
            numD = ps.tile([CW, M], f32, tag="a6", name="acc_numd")

            # ---------------- offset schedules ----------------
            # first/last offsets must span the full row range (full-bank
            # start=True zeroing / stop=True group close): put dy=0 at ends.
            def mk_offsets(r):
                offs = [(dy, dx) for dx in range(-r, r + 1)
                        for dy in range(-r, r + 1)]
                offs.remove((0, -r)); offs.remove((0, r))
                return [(0, -r)] + offs + [(0, r)]

            offsB = mk_offsets(RB)
            offsD = mk_offsets(RD)

            # ---------------- wB: 11x11 bilateral on Xbase ----------------
            nB = len(offsB)
            for i, (dy, dx) in enumerate(offsB):
                t = (dy + RB) * 11 + (dx + RB)
                lo, hi = max(0, -dy), M - max(0, dy)
                L = hi - lo
                st, sp = (i == 0), (i == nB - 1)
                xbs = BS[dx][:, lo + dy:hi + dy]
                ybs = BS[dx][:, M + lo + dy:M + hi + dy]
                xbc = B_b[:, lo:hi]
                d = wrk.tile([CW, L], bf16, tag="d", name=f"d_{i}")
                q = wrk.tile([CW, L], bf16, tag="q", name=f"q_{i}")
                w = wrk.tile([CW, L], bf16, tag="w", name=f"w_{i}")
                p1 = wrk.tile([CW, L], bf16, tag="p1", name=f"p1_{i}")
                m2 = wrk.tile([CW, L], bf16, tag="m2", name=f"m2_{i}")
                n = wrk.tile([CW, L], bf16, tag="n", name=f"n_{i}")
                c2 = wrk.tile([CW, L], bf16, tag="c2", name=f"c2_{i}")
                eng, _ = bal.pick_tt()
                eng.tensor_tensor(d[:], xbs, xbc, Alu.subtract)
                eng, which = bal.pick_tt(allow_act=True)
                if which == "act":
                    nc.scalar.activation(q[:], d[:], Act.Square, scale=sqc[:])
                    nc.scalar.activation(w[:], q[:], Act.Exp, scale=-1.0,
                                         bias=biasB[:, t:t + 1])
                else:
                    eng.tensor_tensor(q[:], d[:], d[:], Alu.mult)
                    nc.scalar.activation(w[:], q[:], Act.Exp, scale=negc[:],
                                         bias=biasB[:, t:t + 1])
                bal.charge("act", bal.ACT)  # the exp
                eng, _ = bal.pick_tt()
                eng.tensor_tensor(p1[:], w[:], d[:], Alu.mult)
                eng, _ = bal.pick_tt()
                eng.tensor_tensor(m2[:], p1[:], d[:], Alu.mult)
                eng, _ = bal.pick_tt()
                eng.tensor_tensor(n[:], w[:], ybs, Alu.mult)
                eng, _ = bal.pick_tt()
                eng.tensor_tensor(c2[:], p1[:], ybs, Alu.mult)
                nc.tensor.matmul(den[:, lo:hi], ident[:], w[:], start=st, stop=sp)
                nc.tensor.matmul(M1[:, lo:hi], ident[:], p1[:], start=st, stop=sp)
                nc.tensor.matmul(M2[:, lo:hi], ident[:], m2[:], start=st, stop=sp)
                nc.tensor.matmul(NN[:, lo:hi], ident[:], n[:], start=st, stop=sp)
                nc.tensor.matmul(C2[:, lo:hi], ident[:], c2[:], start=st, stop=sp)

            # ---------------- wD: 5x5 bilateral on Xdet ----------------
            nD = len(offsD)
            for i, (dy, dx) in enumerate(offsD):
                t = (dy + RD) * 5 + (dx + RD)
                lo, hi = max(0, -dy), M - max(0, dy)
                L = hi - lo
                st, sp = (i == 0), (i == nD - 1)
                xds = DS[dx][:, lo + dy:hi + dy]
                zs = DS[dx][:, M + lo + dy:M + hi + dy]
                xdc = D_b[:, lo:hi]
                d = wrk.tile([CW, L], bf16, tag="dd", name=f"dd_{i}")
                q = wrk.tile([CW, L], bf16, tag="dq", name=f"dq_{i}")
                w = wrk.tile([CW, L], bf16, tag="dw", name=f"dw_{i}")
                tz = wrk.tile([CW, L], bf16, tag="dtz", name=f"dtz_{i}")
                eng, _ = bal.pick_tt()
                eng.tensor_tensor(d[:], xds, xdc, Alu.subtract)
                eng, which = bal.pick_tt(allow_act=True)
                if which == "act":
                    nc.scalar.activation(q[:], d[:], Act.Square, scale=sqc[:])
                    nc.scalar.activation(w[:], q[:], Act.Exp, scale=-1.0,
                                         bias=biasD[:, t:t + 1])
                else:
                    eng.tensor_tensor(q[:], d[:], d[:], Alu.mult)
                    nc.scalar.activation(w[:], q[:], Act.Exp, scale=negc[:],
                                         bias=biasD[:, t:t + 1])
                bal.charge("act", bal.ACT)
                eng, _ = bal.pick_tt()
                eng.tensor_tensor(tz[:], w[:], zs, Alu.mult)
                nc.tensor.matmul(denD[:, lo:hi], ident[:], w[:], start=st, stop=sp)
                nc.tensor.matmul(numD[:, lo:hi], ident[:], tz[:], start=st, stop=sp)

            # ---------------- final assembly (f32) ----------------
            asm = per
            rden = asm.tile([CW, M], f32, name="rden", tag="rden")
            m1 = asm.tile([CW, M], f32, name="m1", tag="m1")
            nn = asm.tile([CW, M], f32, name="nn", tag="nn")
            m2f = asm.tile([CW, M], f32, name="m2f", tag="m2f")
            c2f = asm.tile([CW, M], f32, name="c2f", tag="c2f")
            nc.vector.reciprocal(rden[:], den[:])
            nc.vector.tensor_tensor(m1[:], M1[:], rden[:], Alu.mult)
            nc.vector.tensor_tensor(nn[:], NN[:], rden[:], Alu.mult)
            nc.vector.tensor_tensor(m2f[:], M2[:], rden[:], Alu.mult)
            nc.vector.tensor_tensor(c2f[:], C2[:], rden[:], Alu.mult)
            mm = asm.tile([CW, M], f32, name="mm", tag="mm")
            vx = asm.tile([CW, M], f32, name="vx", tag="vx")
            mn = asm.tile([CW, M], f32, name="mn", tag="mn")
            cxy = asm.tile([CW, M], f32, name="cxy", tag="cxy")
            nc.vector.tensor_tensor(mm[:], m1[:], m1[:], Alu.mult)
            nc.vector.tensor_tensor(vx[:], m2f[:], mm[:], Alu.subtract)
            nc.gpsimd.tensor_tensor(mn[:], m1[:], nn[:], Alu.mult)
            nc.vector.tensor_tensor(cxy[:], c2f[:], mn[:], Alu.subtract)
            vx1 = asm.tile([CW, M], f32, name="vx1", tag="vx1")
            rvx = asm.tile([CW, M], f32, name="rvx", tag="rvx")
            A = asm.tile([CW, M], f32, name="A", tag="A")
            am1 = asm.tile([CW, M], f32, name="am1", tag="am1")
            nc.vector.tensor_scalar_add(vx1[:], vx[:], 1e-6)
            nc.vector.reciprocal(rvx[:], vx1[:])
            nc.vector.tensor_tensor(A[:], cxy[:], rvx[:], Alu.mult)
            nc.vector.tensor_tensor(am1[:], A[:], m1[:], Alu.mult)
            o2 = asm.tile([CW, M], f32, name="o2", tag="o2")
            o3 = asm.tile([CW, M], f32, name="o3", tag="o3")
            nc.vector.tensor_tensor(o2[:], nn[:], am1[:], Alu.subtract)
            nc.gpsimd.tensor_tensor(o3[:], o2[:], xd_f[:], Alu.add)
            rdd = asm.tile([CW, M], f32, name="rdd", tag="rdd")
            bd = asm.tile([CW, M], f32, name="bd", tag="bd")
            outf = asm.tile([CW, M], f32, name="outf", tag="outf")
            nc.vector.reciprocal(rdd[:], denD[:])
            nc.vector.tensor_tensor(bd[:], numD[:], rdd[:], Alu.mult)
            nc.vector.tensor_tensor(outf[:], o3[:], bd[:], Alu.add)
            nc.sync.dma_start(d_out[:], outf[:])

    nc.compile()
    nc._balance_estimate = dict(bal.t)
    return nc


def _get_program():
    if "nc" not in _PROGRAM_CACHE:
        _PROGRAM_CACHE["nc"] = _build_program()
    return _PROGRAM_CACHE["nc"]


def prepare_in_maps(X, y, r):
    """Host-side sharding + parameter tables. Returns list of per-core dicts."""
    X = np.asarray(X, dtype=np.float32)
    y = np.asarray(y, dtype=np.float32)
    r = np.float32(np.asarray(r))
    Xi = X[0, 0]
    yi = y[0, 0]
    sigma = r * (yi.max() - yi.min())
    c = np.float32(1.0) / np.float32((sigma / np.float32(2.0)) ** 2)
    sqc_val = np.float32(math.sqrt(c))

    XT = np.ascontiguousarray(Xi.T)   # [N, M] = [col, row]
    yT = np.ascontiguousarray(yi.T)

    # padded transposed images for halo extraction
    XTp = np.zeros((N + 2 * HW_, M), np.float32)
    XTp[HW_:HW_ + N] = XT
    yTp = np.zeros((N + 2 * HW_, M), np.float32)
    yTp[HW_:HW_ + N] = yT

    ident = np.eye(CW, dtype=np.float32)

    # Toeplitz band matrices for horizontal 5x5 box (with 1/25 folded in).
    # halo partition hp: hp<HW_ -> col c0-HW_+hp ; hp>=HW_ -> col c0+CW+(hp-HW_)
    halo_rel = np.array([(-HW_ + hp) if hp < HW_ else (CW + hp - HW_)
                         for hp in range(2 * HW_)])
    tmm = np.zeros((CW, CW), np.float32)
    thm = np.zeros((2 * HW_, CW), np.float32)
    tmh = np.zeros((CW, 2 * HW_), np.float32)
    thh = np.zeros((2 * HW_, 2 * HW_), np.float32)
    for m in range(CW):
        for k in range(CW):
            if abs(k - m) <= 2:
                tmm[k, m] = 1.0 / 25.0
        for k in range(2 * HW_):
            if abs(halo_rel[k] - m) <= 2:
                thm[k, m] = 1.0 / 25.0
    for hp in range(2 * HW_):
        mcol = halo_rel[hp]
        for k in range(CW):
            if abs(k - mcol) <= 2:
                tmh[k, hp] = 1.0 / 25.0
        for k in range(2 * HW_):
            if abs(halo_rel[k] - mcol) <= 2:
                thh[k, hp] = 1.0 / 25.0

    in_maps = []
    for core in range(NCORES):
        c0 = core * CW
        xm = XTp[HW_ + c0:HW_ + c0 + CW]
        ym_ = yTp[HW_ + c0:HW_ + c0 + CW]
        xh = np.concatenate([XTp[c0:c0 + HW_],
                             XTp[HW_ + c0 + CW:2 * HW_ + c0 + CW]], axis=0)
        yh = np.concatenate([yTp[c0:c0 + HW_],
                             yTp[HW_ + c0 + CW:2 * HW_ + c0 + CW]], axis=0)

        cols = c0 + np.arange(CW)
        biasB = np.zeros((CW, 121), np.float32)
        for dy in range(-RB, RB + 1):
            for dx in range(-RB, RB + 1):
                t = (dy + RB) * 11 + (dx + RB)
                sp = -(dy * dy + dx * dx) / DEN_B
                valid = (cols + dx >= 0) & (cols + dx < N)
                biasB[:, t] = np.where(valid, sp, MASK_BIAS)
        biasD = np.zeros((CW, 25), np.float32)
        for dy in range(-RD, RD + 1):
            for dx in range(-RD, RD + 1):
                t = (dy + RD) * 5 + (dx + RD)
                sp = -(dy * dy + dx * dx) / DEN_D
                valid = (cols + dx >= 0) & (cols + dx < N)
                biasD[:, t] = np.where(valid, sp, MASK_BIAS)

        in_maps.append({
            "xym": np.ascontiguousarray(np.concatenate([xm, ym_], axis=1)),
            "xyh": np.ascontiguousarray(np.concatenate([xh, yh], axis=1)),
            "biasB": biasB,
            "biasD": biasD,
            "sqc": np.full((CW, 1), sqc_val, np.float32),
            "negc": np.full((CW, 1), -c, np.float32),
            "ident": ident,  # cast to bf16 at transfer by caller if needed
            "tmm": tmm, "thm": thm, "tmh": tmh, "thh": thh,
        })
    return in_maps


def _cast_in_maps(in_maps):
    out = []
    for m in in_maps:
        m = dict(m)
        import ml_dtypes
        m["ident"] = m["ident"].astype(ml_dtypes.bfloat16)
        out.append(m)
    return out


def gather_output(results):
    """results: list (per core) of dicts with 'outT' [CW, M]."""
    outT = np.concatenate([np.asarray(res["outT"]) for res in results], axis=0)
    return np.ascontiguousarray(outT.T)[None, None].astype(np.float32)


def kernel(X, y, r):
    from concourse import bass_utils
    nc = _get_program()
    in_maps = _cast_in_maps(prepare_in_maps(X, y, r))
    res = bass_utils.run_bass_kernel_spmd(nc, in_maps,
                                          core_ids=list(range(NCORES)))
    return gather_output(res.results)
